# revision 1
# baseline (speedup 1.0000x reference)
"""CTC-style forward-algorithm loss kernel for Trainium2 (8 NeuronCores, data parallel).

Algorithm (per core, 32 batches):
  The reference DP  fwd_t[s] = logaddexp(fwd_{t-1}[s] + xt4, fwd_{t-1}[s-1] + xt[k_{s-1}])
  is reformulated in probability space relative to per-(chunk, column) scales:
  column-by-column over s (500 sequential steps), where each column update over
  all t is ONE tensor_tensor_scan (affine scan along the free axis) plus one
  multiply.  The t axis (2000) is split into 4 chunks living on partition
  groups (4 chunks x 32 batches = 128 partitions) processed in a wavefront:
  at wavefront step sigma, chunk c works on column s = sigma - c.

  Numerical range is controlled by a per-column drain rate beta (the scan's
  decay coefficient, updated every 16 columns from the measured column growth)
  and per-column renormalization by the column's chunk-end value z.  The
  log-scale bookkeeping (Pacc/LT chains) recovers exact log values; the output
  is read entirely from the LT chain of the last chunk group.
"""

import numpy as np
from contextlib import ExitStack

import concourse.bass as bass
import concourse.tile as tile
import concourse.mybir as mybir
from concourse.bass_utils import run_bass_kernel_spmd

NT, NB, NF, NS = 2000, 256, 5, 500
M = 8                 # cores
B = NB // M           # 32 local batches
NC = 4                # t-chunk groups on partitions
TC = NT // NC         # 500
GRP = 16              # beta update period (columns)
SIG = NS + NC         # 504 wavefront compute steps (sigma = 0..503), +1 finalize
ZFLOOR = 1e-30
F32 = mybir.dt.float32
AL = mybir.AluOpType
AF = mybir.ActivationFunctionType

_CACHE = {}


def _split_multi_waits(nc, max_inline=1):
    """walrus codegen allows only a tiny number of fused sem-waits per TPB
    instruction (Tile's native encoder is more permissive).  Hoist excess
    waits onto InstEventSemaphore carriers on the same engine stream."""
    f = nc.m.functions[0]
    n_split = 0
    for bb in f.blocks:
        out = []
        changed = False
        for inst in bb.instructions:
            si = inst.sync_info
            waits = list(si.on_wait) if si is not None and si.on_wait else []
            if isinstance(inst, mybir.InstEventSemaphore) and len(waits) > 2:
                keep, extra = waits[:2], waits[2:]
            elif not isinstance(inst, mybir.InstEventSemaphore) \
                    and len(waits) > max_inline:
                keep, extra = waits[:max_inline], waits[max_inline:]
            else:
                keep, extra = waits, []
            if extra:
                # EventSemaphore carriers hold <= 2 waits each (walrus limit)
                for j in range(0, len(extra), 2):
                    evt = mybir.InstEventSemaphore(
                        name=f"evw{j}_{inst.name}", ins=[], outs=[])
                    evt.engine = inst.engine
                    evt.sync_info = mybir.SyncInfo(
                        on_wait=extra[j:j + 2], on_update=[])
                    out.append(evt)
                inst.sync_info = mybir.SyncInfo(
                    on_wait=keep, on_update=list(si.on_update))
                changed = True
                n_split += 1
            out.append(inst)
        if changed:
            bb.instructions = out
    return n_split


def build_module(split_waits=True):
    key = ("nc", split_waits)
    if key in _CACHE:
        return _CACHE[key], _CACHE["names"]
    nc = bass.Bass(debug=False)

    erows_d = nc.dram_tensor("erows", [128, 4 * TC], F32, kind="ExternalInput")
    masks_d = nc.dram_tensor("masks", [128, 3 * 512], F32, kind="ExternalInput")
    iot_d = nc.dram_tensor("iot", [128, 512], F32, kind="ExternalInput")
    perm_d = nc.dram_tensor("perm", [128, 128], F32, kind="ExternalInput")
    lout_d = nc.dram_tensor("lout", [32, 512], F32, kind="ExternalOutput")

    with tile.TileContext(nc) as tc, \
         tc.tile_pool(name="state", bufs=1) as st:
        erows = st.tile([128, 4 * TC], F32, tag="erows_sb", name="erows_sb")
        masks = st.tile([128, 3 * 512], F32, tag="masks_sb", name="masks_sb")
        iot = st.tile([128, 512], F32, tag="iot_sb", name="iot_sb")
        perm = st.tile([128, 128], F32, tag="perm_sb", name="perm_sb")
        bufA = st.tile([128, TC + 1], F32, tag="bufA", name="bufA")
        bufB = st.tile([128, TC + 1], F32, tag="bufB", name="bufB")
        d0row = st.tile([128, TC], F32, tag="d0row", name="d0row")
        ramp = st.tile([128, TC + 1], F32, tag="ramp", name="ramp")
        loutT = st.tile([128, 512], F32, tag="loutT", name="loutT")
        # small per-partition scalars
        Pacc = st.tile([128, 1], F32, tag="Pacc", name="Pacc")
        beta = st.tile([128, 1], F32, tag="beta", name="beta")
        nbeta = st.tile([128, 1], F32, tag="nbeta", name="nbeta")
        b500 = st.tile([128, 1], F32, tag="b500", name="b500")
        lz = st.tile([128, 1], F32, tag="lz", name="lz")
        lzmb = st.tile([128, 1], F32, tag="lzmb", name="lzmb")
        zf = st.tile([128, 1], F32, tag="zf", name="zf")
        rz = st.tile([128, 1], F32, tag="rz", name="rz")
        lt = st.tile([128, 1], F32, tag="lt", name="lt")
        hsub = st.tile([128, 1], F32, tag="hsub", name="hsub")
        dbc = st.tile([128, 1], F32, tag="dbc", name="dbc")
        dneg = st.tile([128, 1], F32, tag="dneg", name="dneg")

        nc.sync.dma_start(out=erows[:, :], in_=erows_d.ap()[:, :])
        nc.sync.dma_start(out=masks[:, :], in_=masks_d.ap()[:, :])
        nc.sync.dma_start(out=iot[:, :], in_=iot_d.ap()[:, :])
        nc.sync.dma_start(out=perm[:, :], in_=perm_d.ap()[:, :])

        for t in (bufA, bufB, Pacc, beta, nbeta, b500, lz, loutT):
            nc.vector.memset(t[:, :], 0.0)
        nc.vector.memset(d0row[:, :], 1.0)
        # touch each DMA'd tile once so later consumers inherit the DMA
        # dependency via engine order (ISA wait-slot limit is tiny)
        scr = st.tile([128, 1], F32, tag="scr", name="scr")
        for tl in (erows, masks, iot, perm):
            nc.vector.tensor_copy(scr[:, 0:1], tl[:, 0:1])
        nc.scalar.copy(scr[:, 0:1], erows[:, 0:1])
        nc.scalar.copy(scr[:, 0:1], iot[:, 0:1])

        E0 = erows[:, 0:TC]
        F1 = erows[:, TC:2 * TC]
        F2 = erows[:, 2 * TC:3 * TC]
        F3 = erows[:, 3 * TC:4 * TC]

        with tc.tile_pool(name="esel", bufs=3) as esel_pool, \
             tc.tile_pool(name="upool", bufs=3) as u_pool, \
             tc.tile_pool(name="psum", bufs=2, space="PSUM") as psum_pool:
            for sig in range(SIG + 1):
                cur, prv = (bufA, bufB) if sig % 2 == 0 else (bufB, bufA)
                if sig >= 1:
                    # ---- phase 1: finalize columns s_prev = sig-1-c
                    nc.vector.tensor_scalar_max(zf[:, 0:1], prv[:, TC:TC + 1], ZFLOOR)
                    nc.vector.reciprocal(rz[:, 0:1], zf[:, 0:1])
                    nc.scalar.activation(lz[:, 0:1], zf[:, 0:1], AF.Ln)
                    nc.scalar.mul(prv[:, 0:TC + 1], prv[:, 0:TC + 1], rz[:, 0:1])
                    nc.vector.tensor_sub(lzmb[:, 0:1], lz[:, 0:1], beta[:, 0:1])
                    nc.vector.tensor_add(Pacc[:, 0:1], Pacc[:, 0:1], lzmb[:, 0:1])
                    nc.vector.tensor_add(lt[:, 0:1], Pacc[:, 0:1], b500[:, 0:1])
                    s3 = sig - 1 - (NC - 1)
                    if 0 <= s3 <= NS:
                        nc.scalar.copy(loutT[96:128, s3:s3 + 1], lt[96:128, 0:1])
                if sig == SIG:
                    break
                # ---- phase 2: beta update every GRP columns
                if sig % GRP == 0 and sig > 0:
                    nc.vector.tensor_scalar(dbc[:, 0:1], lz[:, 0:1], 40.0, -40.0,
                                            AL.min, AL.max)
                    nc.vector.scalar_tensor_tensor(beta[:, 0:1], dbc[:, 0:1],
                                                   1.0 / TC, beta[:, 0:1],
                                                   AL.mult, AL.add)
                    nc.vector.scalar_tensor_tensor(nbeta[:, 0:1], dbc[:, 0:1],
                                                   -1.0 / TC, nbeta[:, 0:1],
                                                   AL.mult, AL.add)
                    nc.vector.tensor_add(b500[:, 0:1], b500[:, 0:1], dbc[:, 0:1])
                    nc.vector.tensor_scalar_mul(dneg[:, 0:1], dbc[:, 0:1], -1.0 / TC)
                    nc.scalar.activation(ramp[:, 0:TC + 1], iot[:, 0:TC + 1], AF.Exp,
                                         scale=dneg[:, 0:1])
                    nc.vector.tensor_mul(prv[:, 0:TC + 1], prv[:, 0:TC + 1],
                                         ramp[:, 0:TC + 1])
                    nc.scalar.activation(d0row[:, 0:TC], iot[:, 0:TC], AF.Exp,
                                         scale=0.0, bias=nbeta[:, 0:1])
                # ---- phase 3: halo/init writes into cur[:, 0]
                if sig == 0:
                    nc.vector.memset(cur[0:32, 0:1], 1.0)
                elif sig in (1, 2):
                    nc.vector.memset(cur[0:32, 0:1], 0.0)
                if sig in (1, 2, 3):
                    # group `sig` is born this step: its Pacc must start at 0
                    # (full-width phase-1 ops polluted it with log(ZFLOOR))
                    nc.vector.memset(Pacc[32 * sig:32 * sig + 32, 0:1], 0.0)
                if sig >= 1:
                    # cross-quadrant shift of lt by +32 partitions via a PE
                    # permutation matmul (compute engines can't mix partition
                    # bases within one instruction); then aligned ops only.
                    ltsh = psum_pool.tile([128, 1], F32, tag="ltsh")
                    nc.tensor.matmul(ltsh[:, 0:1], perm[:, :], lt[:, 0:1],
                                     start=True, stop=True)
                    nc.vector.tensor_sub(hsub[:, 0:1], ltsh[:, 0:1], Pacc[:, 0:1])
                    # 32/96-start APs span <= 32 partitions -> per-quadrant ops
                    for q in (1, 2, 3):
                        a, b = 32 * q, 32 * q + 32
                        nc.scalar.activation(cur[a:b, 0:1], hsub[a:b, 0:1], AF.Exp,
                                             scale=1.0, bias=beta[a:b, 0:1])
                    # pre-birth groups must keep zero feeds: overwrite their
                    # halo slots (the full-width halo writes exp(garbage)=1s)
                    for c in range(sig + 1, NC):
                        nc.vector.memset(cur[32 * c:32 * c + 32, 0:1], 0.0)
                # ---- phase 4: select + mul + scan
                a1 = esel_pool.tile([128, TC], F32, tag="a1")
                a2 = esel_pool.tile([128, TC], F32, tag="a2")
                a3 = esel_pool.tile([128, TC], F32, tag="a3")
                nc.vector.scalar_tensor_tensor(a1[:, :], F1,
                                               masks[:, 0 * 512 + sig:0 * 512 + sig + 1],
                                               E0, AL.mult, AL.add)
                nc.vector.scalar_tensor_tensor(a2[:, :], F2,
                                               masks[:, 1 * 512 + sig:1 * 512 + sig + 1],
                                               a1[:, :], AL.mult, AL.add)
                nc.vector.scalar_tensor_tensor(a3[:, :], F3,
                                               masks[:, 2 * 512 + sig:2 * 512 + sig + 1],
                                               a2[:, :], AL.mult, AL.add)
                u = u_pool.tile([128, TC], F32, tag="u")
                nc.vector.tensor_mul(u[:, :], a3[:, :], prv[:, 0:TC])
                nc.vector.tensor_tensor_scan(cur[:, 1:TC + 1], d0row[:, :], u[:, :],
                                             cur[:, 0:1], AL.mult, AL.add)

        nc.sync.dma_start(out=lout_d.ap()[:, :], in_=loutT[96:128, :])

    if split_waits:
        _split_multi_waits(nc)

    _CACHE[key] = nc
    _CACHE["names"] = dict(ins=["erows", "masks", "iot", "perm"], out="lout")
    return nc, _CACHE["names"]


def host_prep(x, seqs):
    """Build per-core input arrays. Returns list of dicts."""
    f32 = np.float32
    x8 = np.ascontiguousarray(x.reshape(NT, M, B, NF).astype(f32))
    D = x8[..., :4] - x8[..., 4:5]                       # (NT, M, B, 4)
    E = np.exp(D, dtype=f32)
    Ecb = E.reshape(NC, TC, M, B, 4).transpose(2, 0, 3, 4, 1)  # (M, NC, B, 4, TC)
    E0 = Ecb[..., 0, :]
    sq = seqs.reshape(M, B, NS)

    iot = np.broadcast_to(np.arange(512, dtype=f32), (128, 512)).copy()
    # shift-by-32 permutation: ltsh = perm.T @ lt, ltsh[q+32] = lt[q]
    perm = np.zeros((128, 128), dtype=f32)
    for q in range(96):
        perm[q, q + 32] = 1.0

    in_maps = []
    for m in range(M):
        er = np.empty((128, 4 * TC), dtype=f32)
        for c in range(NC):
            p = slice(32 * c, 32 * c + 32)
            er[p, 0:TC] = E0[m, c]
            er[p, TC:2 * TC] = Ecb[m, c, :, 1, :] - E0[m, c]
            er[p, 2 * TC:3 * TC] = Ecb[m, c, :, 2, :] - E0[m, c]
            er[p, 3 * TC:4 * TC] = Ecb[m, c, :, 3, :] - E0[m, c]
        mk = np.zeros((128, 3 * 512), dtype=f32)
        s_arr = np.arange(1, NS + 1)
        for c in range(NC):
            p = slice(32 * c, 32 * c + 32)
            for ki, k in enumerate((1, 2, 3)):
                blk = np.zeros((B, 512), dtype=f32)
                blk[:, s_arr + c] = (sq[m, :, s_arr - 1].T == k)
                mk[p, 512 * ki:512 * (ki + 1)] = blk
        in_maps.append({"erows": er, "masks": mk, "iot": iot, "perm": perm})
    return in_maps


def host_post(x, seqlens, louts):
    f32 = np.float32
    x8 = x.reshape(NT, M, B, NF)
    C2000 = x8[:, :, :, 4].sum(axis=0, dtype=np.float32)   # (M, B)
    lt3 = np.stack([louts[m][:, 0:NS + 1] for m in range(M)])  # (M, B, NS+1)
    fwd = lt3 + C2000[:, :, None]
    fwd = fwd.reshape(NB, NS + 1)
    out = -np.take_along_axis(fwd, seqlens[:, None].astype(np.int64), axis=1) / f32(NT)
    return out.astype(np.float32)


def kernel(x, seqs, seqlens):
    nc, names = build_module()
    in_maps = host_prep(np.asarray(x), np.asarray(seqs))
    res = run_bass_kernel_spmd(nc, in_maps, list(range(M)))
    louts = [res.results[m]["lout"] for m in range(M)]
    return host_post(np.asarray(x), np.asarray(seqlens), louts)



# revision 5
# speedup vs baseline: 1.5956x; 1.5956x over previous
"""CTC forward-loss kernel for Trainium2, 8 cores data-parallel (32 batch/core).

v2 architecture (validated against a numpy prototype):
  Layout: 128 partitions = 4 t-chunks x 32 batches; free axis = t within
  chunk (500).  Skew-2 wavefront over columns s: chunk c scans column
  s = sigma - 2c at step sigma (507 steps).

  Per sigma:
    PE    4 select-matmuls W_s.T @ X_c -> PSUM produce the move coefficients
          b[t] = E[t, seqs[s-1]] for each chunk (one-hot W, fp16 tables),
          issued 2 sigmas ahead; plus a tiny halo-shift matmul.
    DVE   u = (b * d) * prv   (scalar_tensor_tensor from PSUM)
          cur[1:] = scan(d * y_prev + u)  (tensor_tensor_scan, fp32 state)
    ACT   init-slot copy (halo -> cur[:,0]) and chunk-3 output staging.

  Numerics: stored values are G * exp(-beta_c(epoch) * t_local) * renorm,
  with a compile-time beta schedule (binomial-slope model) and a shared
  per-batch renorm every R=8 sigmas measured from the full-buffer sum.
  Epoch transitions re-ramp the state on device (ACT exp of iota).  The
  host replays the schedule + logged renorm factors to undo all scales.
"""

import math
import numpy as np

import concourse.bass as bass
import concourse.tile as tile
import concourse.mybir as mybir
from concourse.bass_utils import run_bass_kernel_spmd

NT, NB, NF, NS = 2000, 256, 5, 500
M = 8
B = NB // M           # 32 local batches
NC, TC = 4, 500
SKEW = 2
SIG = NS + SKEW * (NC - 1) + 1          # 507 wavefront steps
R = 8                                    # renorm/epoch cadence
NEP = 80                                 # schedule table width (>= SIG//R + 2)
LA = 2                                   # select lookahead (psum bufs = 4)
ZTINY = 1e-30
F32 = mybir.dt.float32
F16 = mybir.dt.float16
AL = mybir.AluOpType
AF = mybir.ActivationFunctionType
AX = mybir.AxisListType

_CACHE = {}


# ---------------------------------------------------------------- schedule --

def _lnC(n, k):
    if k < 0 or k > n:
        return -math.inf
    return (math.lgamma(n + 1) - math.lgamma(k + 1) - math.lgamma(n - k + 1))


def _beta_model(c, s):
    t0, t1 = 500 * c, 500 * (c + 1)
    s = min(s, t1 - 64)
    tlo = max(t0, s + 1)
    if t1 <= tlo + 1:
        return 0.0
    num = _lnC(t1, s) - _lnC(tlo, s)
    if not math.isfinite(num):
        return 0.0
    return num / (t1 - tlo)


def beta_schedule():
    """BETA[c, ep], monotone non-decreasing per chunk."""
    n_ep = NEP
    BETA = np.zeros((NC, n_ep))
    for c in range(NC):
        prev = 0.0
        for ep in range(n_ep):
            smid = min(max(ep * R + R // 2 - SKEW * c, 0), NS)
            b = max(_beta_model(c, smid), prev)
            BETA[c, ep] = b
            prev = b
    return BETA


# ---------------------------------------------------------------- module ----

def _split_multi_waits(nc, max_inline=1):
    """walrus allows few fused sem-waits per instruction; hoist extras onto
    EventSemaphore carriers on the same engine stream."""
    f = nc.m.functions[0]
    for bb in f.blocks:
        out = []
        changed = False
        for inst in bb.instructions:
            si = inst.sync_info
            waits = list(si.on_wait) if si is not None and si.on_wait else []
            if isinstance(inst, mybir.InstEventSemaphore) and len(waits) > 2:
                keep, extra = waits[:2], waits[2:]
            elif not isinstance(inst, mybir.InstEventSemaphore) \
                    and len(waits) > max_inline:
                keep, extra = waits[:max_inline], waits[max_inline:]
            else:
                keep, extra = waits, []
            if extra:
                for j in range(0, len(extra), 2):
                    evt = mybir.InstEventSemaphore(
                        name=f"evw{j}_{inst.name}", ins=[], outs=[])
                    evt.engine = inst.engine
                    evt.sync_info = mybir.SyncInfo(
                        on_wait=extra[j:j + 2], on_update=[])
                    out.append(evt)
                inst.sync_info = mybir.SyncInfo(
                    on_wait=keep, on_update=list(si.on_update))
                changed = True
            out.append(inst)
        if changed:
            bb.instructions = out


def build_module(split_waits=True):
    key = ("nc", split_waits)
    if key in _CACHE:
        return _CACHE[key], _CACHE["names"]
    nc = bass.Bass(debug=False)

    xtab_d = nc.dram_tensor("xtab", [128, NC * TC], F16, kind="ExternalInput")
    wtab_d = nc.dram_tensor("wtab", [128, (NS + 1) * 32], F16, kind="ExternalInput")
    mats_d = nc.dram_tensor("mats", [128, 256], F32, kind="ExternalInput")
    sched_d = nc.dram_tensor("sched", [128, 4 * NEP], F32, kind="ExternalInput")
    iot_d = nc.dram_tensor("iot", [128, 512], F32, kind="ExternalInput")
    out_d = nc.dram_tensor("outst", [32, 512], F32, kind="ExternalOutput")
    rz_d = nc.dram_tensor("rzst", [128, NEP], F32, kind="ExternalOutput")

    with tile.TileContext(nc) as tc, \
         tc.tile_pool(name="state", bufs=1) as st:
        xtab = st.tile([128, NC * TC], F16, tag="xtab", name="xtab_sb")
        wtab = st.tile([128, (NS + 1) * 32], F16, tag="wtab", name="wtab_sb")
        mats = st.tile([128, 256], F32, tag="mats", name="mats_sb")
        sched = st.tile([128, 4 * NEP], F32, tag="sched", name="sched_sb")
        iot = st.tile([128, 512], F32, tag="iot", name="iot_sb")
        bufA = st.tile([128, TC + 1], F32, tag="bufA", name="bufA")
        bufB = st.tile([128, TC + 1], F32, tag="bufB", name="bufB")
        u = st.tile([128, TC + 1], F32, tag="u", name="u")
        d0row = st.tile([128, TC], F32, tag="d0row", name="d0row")
        ramp = st.tile([128, TC + 1], F32, tag="ramp", name="ramp")
        ones = st.tile([128, TC + 1], F32, tag="ones", name="ones")
        outst = st.tile([128, 512], F32, tag="outst", name="outst")
        rzst = st.tile([128, NEP], F32, tag="rzst", name="rzst")
        zred = st.tile([128, 1], F32, tag="zred", name="zred")
        zcl = st.tile([128, 1], F32, tag="zcl", name="zcl")
        rz = st.tile([128, 1], F32, tag="rz", name="rz")
        vsc = st.tile([128, 1], F32, tag="vsc", name="vsc")

        nc.sync.dma_start(out=xtab[:, :], in_=xtab_d.ap()[:, :])
        nc.sync.dma_start(out=wtab[:, :], in_=wtab_d.ap()[:, :])
        nc.sync.dma_start(out=mats[:, :], in_=mats_d.ap()[:, :])
        nc.sync.dma_start(out=sched[:, :], in_=sched_d.ap()[:, :])
        nc.sync.dma_start(out=iot[:, :], in_=iot_d.ap()[:, :])

        DT = sched[:, 0 * NEP:1 * NEP]       # exp(-beta_c(ep))
        DBN = sched[:, 1 * NEP:2 * NEP]      # -(beta_c(ep) - beta_c(ep-1))
        DBI = sched[:, 2 * NEP:3 * NEP]      # -TC * sum_{c'<c} dbeta_{c'}(ep)
        FXB = sched[:, 3 * NEP:4 * NEP]      # exp(-TC*sum_{c'<=c-1} dbeta(ep))
        PERM = mats[:, 0:128]
        GSUM = mats[:, 128:256]

        for t in (bufA, bufB, u, outst, rzst):
            nc.vector.memset(t[:, :], 0.0)
        nc.vector.memset(ones[:, :], 1.0)
        # touch DMA'd tiles once (wait-slot pressure)
        scr = st.tile([128, 1], F32, tag="scr", name="scr")
        nc.vector.tensor_copy(scr[:, 0:1], mats[:, 0:1])
        nc.vector.tensor_copy(scr[:, 0:1], sched[:, 0:1])
        nc.vector.tensor_copy(scr[:, 0:1], iot[:, 0:1])

        # initial d0row for epoch 0
        nc.scalar.mul(d0row[:, 0:TC], ones[:, 0:TC], DT[:, 0:1])

        with tc.tile_pool(name="bsel", bufs=4, space="PSUM") as bp, \
             tc.tile_pool(name="hal", bufs=2, space="PSUM") as hp, \
             tc.tile_pool(name="zs", bufs=2, space="PSUM") as zp:
            bps = [None] * (SIG + LA)
            halo = [None, None]

            def issue_selects(sL):
                bt = bp.tile([128, 512], F32, tag="bsel")
                bps[sL] = bt
                for c in range(NC):
                    s = sL - SKEW * c
                    slot = s if 1 <= s <= NS else 0
                    nc.tensor.matmul(
                        bt[32 * c:32 * c + 32, 0:TC],
                        wtab[:, 32 * slot:32 * slot + 32],
                        xtab[:, TC * c:TC * (c + 1)],
                        start=True, stop=True, tile_position=(0, 32 * c))

            for sL in range(LA):
                issue_selects(sL)

            for sig in range(SIG):
                cur, prv = (bufA, bufB) if sig % 2 == 0 else (bufB, bufA)
                ep = sig // R
                event = (sig % R == 0 and sig > 0)
                if event:
                    # epoch ramp: exp(-dbeta*j - TC*cumdbeta)
                    nc.scalar.activation(ramp[:, 0:TC + 1], iot[:, 0:TC + 1],
                                         AF.Exp, scale=DBN[:, ep:ep + 1],
                                         bias=DBI[:, ep:ep + 1])
                    nc.vector.tensor_mul(prv[:, 0:TC + 1], prv[:, 0:TC + 1],
                                         ramp[:, 0:TC + 1])
                    # shared renorm from full-buffer sum
                    nc.vector.tensor_reduce(zred[:, 0:1], prv[:, 0:TC + 1],
                                            AX.X, AL.add)
                    zsp = zp.tile([128, 1], F32, tag="zs")
                    nc.tensor.matmul(zsp[:, 0:1], GSUM, zred[:, 0:1],
                                     start=True, stop=True)
                    nc.vector.tensor_scalar_max(zcl[:, 0:1], zsp[:, 0:1], ZTINY)
                    nc.vector.reciprocal(rz[:, 0:1], zcl[:, 0:1])
                    nc.scalar.mul(prv[:, 0:TC + 1], prv[:, 0:TC + 1], rz[:, 0:1])
                    nc.vector.tensor_copy(rzst[:, ep:ep + 1], rz[:, 0:1])
                    nc.vector.tensor_mul(vsc[:, 0:1], FXB[:, ep:ep + 1], rz[:, 0:1])
                    # refresh drain row for the new epoch
                    nc.scalar.mul(d0row[:, 0:TC], ones[:, 0:TC], DT[:, ep:ep + 1])

                # select lookahead
                if sig + LA < SIG:
                    issue_selects(sig + LA)

                # init slot: halo (written at sig-2) with event fixes
                if sig == 0:
                    nc.vector.memset(cur[:, 0:1], 0.0)
                    nc.vector.memset(cur[0:32, 0:1], 1.0)
                elif sig == 1:
                    nc.vector.memset(cur[:, 0:1], 0.0)
                else:
                    hsrc = halo[sig % 2]
                    if sig % R in (0, 1) and sig >= R:
                        nc.scalar.activation(cur[:, 0:1], hsrc[:, 0:1], AF.Copy,
                                             scale=vsc[:, 0:1])
                    else:
                        nc.scalar.activation(cur[:, 0:1], hsrc[:, 0:1], AF.Copy)

                # u = (b * d) * prv ; slot 0 of u stays 0
                nc.vector.scalar_tensor_tensor(
                    u[:, 1:TC + 1], bps[sig][:, 0:TC], DT[:, ep:ep + 1],
                    prv[:, 0:TC], AL.mult, AL.mult)
                # scan
                nc.vector.tensor_tensor_scan(
                    cur[:, 1:TC + 1], d0row[:, 0:TC], u[:, 1:TC + 1],
                    cur[:, 0:1], AL.mult, AL.add)

                # halo extraction for sig+2
                if sig + 2 < SIG:
                    hps = hp.tile([128, 1], F32, tag="hal")
                    halo[sig % 2] = hps
                    nc.tensor.matmul(hps[:, 0:1], PERM, cur[:, TC:TC + 1],
                                     start=True, stop=True)

                # chunk-3 output staging
                s3 = sig - SKEW * (NC - 1)
                if 0 <= s3 <= NS:
                    nc.scalar.copy(outst[96:128, s3:s3 + 1],
                                   cur[96:128, TC:TC + 1])

        nc.sync.dma_start(out=out_d.ap()[:, :], in_=outst[96:128, 0:512])
        nc.sync.dma_start(out=rz_d.ap()[:, :], in_=rzst[:, :])

    if split_waits:
        _split_multi_waits(nc)

    _CACHE[key] = nc
    _CACHE["names"] = dict(ins=["xtab", "wtab", "mats", "sched", "iot"],
                           out=["outst", "rzst"])
    return nc, _CACHE["names"]


# ---------------------------------------------------------------- host ------

def host_prep(x, seqs):
    f32, f16 = np.float32, np.float16
    BETA = beta_schedule()
    x = np.asarray(x)
    seqs = np.asarray(seqs)

    # schedule tables, per partition p = 32c + b
    dt_t = np.zeros((128, NEP), f32)
    dbn_t = np.zeros((128, NEP), f32)
    dbi_t = np.zeros((128, NEP), f32)
    fxb_t = np.ones((128, NEP), f32)
    for ep in range(NEP):
        db = BETA[:, ep] - (BETA[:, ep - 1] if ep > 0 else BETA[:, 0])
        cum = np.concatenate([[0.0], np.cumsum(db)[:-1]])    # sum_{c'<c}
        dsum = np.cumsum(db)                                  # sum_{c'<=c}
        for c in range(NC):
            p = slice(32 * c, 32 * c + 32)
            dt_t[p, ep] = np.exp(-BETA[c, ep])
            dbn_t[p, ep] = -db[c]
            dbi_t[p, ep] = -TC * cum[c]
            fxb_t[p, ep] = np.exp(-TC * dsum[c - 1]) if c >= 1 else 1.0
    sched = np.concatenate([dt_t, dbn_t, dbi_t, fxb_t], axis=1)

    iot = np.broadcast_to(np.arange(512, dtype=f32), (128, 512)).copy()

    mats = np.zeros((128, 256), f32)
    for q in range(96):
        mats[q, 32 + q] = 1.0            # perm: out[p] = in[p-32]
    for pi in range(128):
        for c in range(NC):
            mats[pi, 128 + ((pi % 32) + 32 * c) % 128] = 0.0  # placeholder
    # gsum: out[po] = sum_c in[32c + (po%32)]
    gs = np.zeros((128, 128), f32)
    for po in range(128):
        for c in range(NC):
            gs[32 * c + (po % 32), po] = 1.0
    mats[:, 128:256] = gs

    in_maps = []
    for m in range(M):
        xb = x[:, B * m:B * (m + 1), :].astype(f32)           # (NT, 32, 5)
        E = np.exp(xb[:, :, :4] - xb[:, :, 4:5])              # (NT, 32, 4)
        xt = np.empty((128, NC * TC), f16)
        for c in range(NC):
            blk = E[500 * c:500 * (c + 1)]                    # (500, 32, 4)
            for k in range(4):
                xt[32 * k:32 * k + 32, TC * c:TC * (c + 1)] = \
                    blk[:, :, k].T.astype(f16)
        sq = seqs[B * m:B * (m + 1)]                          # (32, NS)
        wt = np.zeros((128, (NS + 1) * 32), f16)
        for s in range(1, NS + 1):
            k = sq[:, s - 1]                                  # (32,)
            wt[32 * k + np.arange(B), 32 * s + np.arange(B)] = 1.0
        in_maps.append({"xtab": xt, "wtab": wt, "mats": mats,
                        "sched": sched, "iot": iot})
    return in_maps


def host_post(x, seqlens, outs, rzs):
    BETA = beta_schedule()
    x = np.asarray(x)
    seqlens = np.asarray(seqlens)
    loss = np.zeros((NB, 1), np.float32)
    for m in range(M):
        raw = outs[m][:, 0:NS + 1].astype(np.float64)         # (32, 501)
        rzv = rzs[m][0:32, :].astype(np.float64)              # rz per batch,epoch
        lnrz = np.zeros((32, SIG))
        for ep in range(1, NEP):
            sg = ep * R
            if sg < SIG:
                lnrz[:, sg] = np.log(np.maximum(rzv[:, ep], 1e-300))
        lnrz_cum = np.cumsum(lnrz, axis=1)
        C2000 = x[:, B * m:B * (m + 1), 4].sum(axis=0, dtype=np.float64)
        fwd = np.zeros((32, NS + 1))
        for s in range(NS + 1):
            sig = s + SKEW * (NC - 1)
            ep = sig // R
            corr = TC * BETA[:, ep].sum() - lnrz_cum[:, sig]
            fwd[:, s] = np.log(np.maximum(raw[:, s], 1e-300)) + corr + C2000
        sl = seqlens[B * m:B * (m + 1)].astype(np.int64)
        loss[B * m:B * (m + 1), 0] = \
            (-np.take_along_axis(fwd, sl[:, None], axis=1) / NT)[:, 0]
    return loss.astype(np.float32)


def kernel(x, seqs, seqlens):
    nc, names = build_module()
    in_maps = host_prep(x, seqs)
    res = run_bass_kernel_spmd(nc, in_maps, list(range(M)))
    outs = [res.results[m]["outst"] for m in range(M)]
    rzs = [res.results[m]["rzst"] for m in range(M)]
    return host_post(x, seqlens, outs, rzs)


# revision 6
# speedup vs baseline: 1.7522x; 1.0981x over previous
"""CTC forward-loss kernel for Trainium2, 8 cores data-parallel (32 batch/core).

v2 architecture (validated against a numpy prototype):
  Layout: 128 partitions = 4 t-chunks x 32 batches; free axis = t within
  chunk (500).  Skew-2 wavefront over columns s: chunk c scans column
  s = sigma - 2c at step sigma (507 steps).

  Per sigma:
    PE    4 select-matmuls W_s.T @ X_c -> PSUM produce the move coefficients
          b[t] = E[t, seqs[s-1]] for each chunk (one-hot W, fp16 tables),
          issued 2 sigmas ahead; plus a tiny halo-shift matmul.
    DVE   u = (b * d) * prv   (scalar_tensor_tensor from PSUM)
          cur[1:] = scan(d * y_prev + u)  (tensor_tensor_scan, fp32 state)
    ACT   init-slot copy (halo -> cur[:,0]) and chunk-3 output staging.

  Numerics: stored values are G * exp(-beta_c(epoch) * t_local) * renorm,
  with a compile-time beta schedule (binomial-slope model) and a shared
  per-batch renorm every R=8 sigmas measured from the full-buffer sum.
  Epoch transitions re-ramp the state on device (ACT exp of iota).  The
  host replays the schedule + logged renorm factors to undo all scales.
"""

import math
import numpy as np

import concourse.bass as bass
import concourse.tile as tile
import concourse.mybir as mybir
from concourse.bass_utils import run_bass_kernel_spmd

NT, NB, NF, NS = 2000, 256, 5, 500
M = 8
B = NB // M           # 32 local batches
NC, TC = 4, 500
SKEW = 2
SIG = NS + SKEW * (NC - 1) + 1          # 507 wavefront steps
R = 8                                    # renorm/epoch cadence
NEP = 80                                 # schedule table width (>= SIG//R + 2)
LA = 2                                   # select lookahead (psum bufs = 4)
ZTINY = 1e-30
F32 = mybir.dt.float32
F16 = mybir.dt.float16
AL = mybir.AluOpType
AF = mybir.ActivationFunctionType
AX = mybir.AxisListType

_CACHE = {}


# ---------------------------------------------------------------- schedule --

def _lnC(n, k):
    if k < 0 or k > n:
        return -math.inf
    return (math.lgamma(n + 1) - math.lgamma(k + 1) - math.lgamma(n - k + 1))


def _beta_model(c, s):
    t0, t1 = 500 * c, 500 * (c + 1)
    s = min(s, t1 - 64)
    tlo = max(t0, s + 1)
    if t1 <= tlo + 1:
        return 0.0
    num = _lnC(t1, s) - _lnC(tlo, s)
    if not math.isfinite(num):
        return 0.0
    return num / (t1 - tlo)


def beta_schedule():
    """BETA[c, ep], monotone non-decreasing per chunk."""
    n_ep = NEP
    BETA = np.zeros((NC, n_ep))
    for c in range(NC):
        prev = 0.0
        for ep in range(n_ep):
            smid = min(max(ep * R + R // 2 - SKEW * c, 0), NS)
            b = max(_beta_model(c, smid), prev)
            BETA[c, ep] = b
            prev = b
    return BETA


# ---------------------------------------------------------------- module ----

def _split_multi_waits(nc, max_inline=1):
    """walrus allows few fused sem-waits per instruction; hoist extras onto
    EventSemaphore carriers on the same engine stream."""
    f = nc.m.functions[0]
    for bb in f.blocks:
        out = []
        changed = False
        for inst in bb.instructions:
            si = inst.sync_info
            waits = list(si.on_wait) if si is not None and si.on_wait else []
            if isinstance(inst, mybir.InstEventSemaphore) and len(waits) > 2:
                keep, extra = waits[:2], waits[2:]
            elif not isinstance(inst, mybir.InstEventSemaphore) \
                    and len(waits) > max_inline:
                keep, extra = waits[:max_inline], waits[max_inline:]
            else:
                keep, extra = waits, []
            if extra:
                for j in range(0, len(extra), 2):
                    evt = mybir.InstEventSemaphore(
                        name=f"evw{j}_{inst.name}", ins=[], outs=[])
                    evt.engine = inst.engine
                    evt.sync_info = mybir.SyncInfo(
                        on_wait=extra[j:j + 2], on_update=[])
                    out.append(evt)
                inst.sync_info = mybir.SyncInfo(
                    on_wait=keep, on_update=list(si.on_update))
                changed = True
            out.append(inst)
        if changed:
            bb.instructions = out


def build_module(split_waits=True):
    key = ("nc", split_waits)
    if key in _CACHE:
        return _CACHE[key], _CACHE["names"]
    nc = bass.Bass(debug=False)

    xtab_d = nc.dram_tensor("xtab", [128, NC * TC], F16, kind="ExternalInput")
    wtab_d = nc.dram_tensor("wtab", [128, (NS + 1) * 32], F16, kind="ExternalInput")
    mats_d = nc.dram_tensor("mats", [128, 256], F32, kind="ExternalInput")
    sched_d = nc.dram_tensor("sched", [128, 4 * NEP], F32, kind="ExternalInput")
    iot_d = nc.dram_tensor("iot", [128, 512], F32, kind="ExternalInput")
    out_d = nc.dram_tensor("outst", [32, 512], F32, kind="ExternalOutput")
    rz_d = nc.dram_tensor("rzst", [128, NEP], F32, kind="ExternalOutput")

    with tile.TileContext(nc) as tc, \
         tc.tile_pool(name="state", bufs=1) as st:
        xtab = st.tile([128, NC * TC], F16, tag="xtab", name="xtab_sb")
        wtab = st.tile([128, (NS + 1) * 32], F16, tag="wtab", name="wtab_sb")
        mats = st.tile([128, 256], F32, tag="mats", name="mats_sb")
        sched = st.tile([128, 4 * NEP], F32, tag="sched", name="sched_sb")
        iot = st.tile([128, 512], F32, tag="iot", name="iot_sb")
        bufA = st.tile([128, TC + 1], F32, tag="bufA", name="bufA")
        bufB = st.tile([128, TC + 1], F32, tag="bufB", name="bufB")
        u = st.tile([128, TC + 1], F32, tag="u", name="u")
        d0row = st.tile([128, TC], F32, tag="d0row", name="d0row")
        ramp = st.tile([128, TC + 1], F32, tag="ramp", name="ramp")
        ones = st.tile([128, TC + 1], F32, tag="ones", name="ones")
        outst = st.tile([128, 512], F32, tag="outst", name="outst")
        rzst = st.tile([128, NEP], F32, tag="rzst", name="rzst")
        zred = st.tile([128, 1], F32, tag="zred", name="zred")
        zcl = st.tile([128, 1], F32, tag="zcl", name="zcl")
        haloSB = st.tile([128, 2], F32, tag="haloSB", name="haloSB")
        rz = st.tile([128, 1], F32, tag="rz", name="rz")
        vsc = st.tile([128, 1], F32, tag="vsc", name="vsc")

        nc.sync.dma_start(out=xtab[:, :], in_=xtab_d.ap()[:, :])
        nc.sync.dma_start(out=wtab[:, :], in_=wtab_d.ap()[:, :])
        nc.sync.dma_start(out=mats[:, :], in_=mats_d.ap()[:, :])
        nc.sync.dma_start(out=sched[:, :], in_=sched_d.ap()[:, :])
        nc.sync.dma_start(out=iot[:, :], in_=iot_d.ap()[:, :])

        DT = sched[:, 0 * NEP:1 * NEP]       # exp(-beta_c(ep))
        DBN = sched[:, 1 * NEP:2 * NEP]      # -(beta_c(ep) - beta_c(ep-1))
        DBI = sched[:, 2 * NEP:3 * NEP]      # -TC * sum_{c'<c} dbeta_{c'}(ep)
        FXB = sched[:, 3 * NEP:4 * NEP]      # exp(-TC*sum_{c'<=c-1} dbeta(ep))
        PERM = mats[:, 0:128]
        GSUM = mats[:, 128:256]

        for t in (bufA, bufB, u, outst, rzst, haloSB):
            nc.vector.memset(t[:, :], 0.0)
        nc.vector.memset(ones[:, :], 1.0)
        # touch DMA'd tiles once (wait-slot pressure)
        scr = st.tile([128, 1], F32, tag="scr", name="scr")
        nc.vector.tensor_copy(scr[:, 0:1], mats[:, 0:1])
        nc.vector.tensor_copy(scr[:, 0:1], sched[:, 0:1])
        nc.vector.tensor_copy(scr[:, 0:1], iot[:, 0:1])

        # initial d0row for epoch 0
        nc.scalar.mul(d0row[:, 0:TC], ones[:, 0:TC], DT[:, 0:1])

        with tc.tile_pool(name="bsel", bufs=4, space="PSUM") as bp, \
             tc.tile_pool(name="zs", bufs=2, space="PSUM") as zp:
            bps = [None] * (SIG + LA)

            def issue_selects(sL):
                bt = bp.tile([128, 512], F32, tag="bsel")
                bps[sL] = bt
                for c in range(NC):
                    s = sL - SKEW * c
                    slot = s if 1 <= s <= NS else 0
                    nc.tensor.matmul(
                        bt[32 * c:32 * c + 32, 0:TC],
                        wtab[:, 32 * slot:32 * slot + 32],
                        xtab[:, TC * c:TC * (c + 1)],
                        start=True, stop=True, tile_position=(0, 32 * c))

            for sL in range(LA):
                issue_selects(sL)

            for sig in range(SIG):
                cur, prv = (bufA, bufB) if sig % 2 == 0 else (bufB, bufA)
                ep = sig // R
                event = (sig % R == 0 and sig > 0)
                if event:
                    # epoch ramp: exp(-dbeta*j - TC*cumdbeta)
                    nc.scalar.activation(ramp[:, 0:TC + 1], iot[:, 0:TC + 1],
                                         AF.Exp, scale=DBN[:, ep:ep + 1],
                                         bias=DBI[:, ep:ep + 1])
                    nc.vector.tensor_mul(prv[:, 0:TC + 1], prv[:, 0:TC + 1],
                                         ramp[:, 0:TC + 1])
                    # shared renorm from full-buffer sum
                    nc.vector.tensor_reduce(zred[:, 0:1], prv[:, 0:TC + 1],
                                            AX.X, AL.add)
                    zsp = zp.tile([128, 1], F32, tag="zs")
                    nc.tensor.matmul(zsp[:, 0:1], GSUM, zred[:, 0:1],
                                     start=True, stop=True)
                    nc.vector.tensor_scalar_max(zcl[:, 0:1], zsp[:, 0:1], ZTINY)
                    nc.vector.reciprocal(rz[:, 0:1], zcl[:, 0:1])
                    nc.scalar.mul(prv[:, 0:TC + 1], prv[:, 0:TC + 1], rz[:, 0:1])
                    nc.scalar.copy(rzst[:, ep:ep + 1], rz[:, 0:1])
                    nc.vector.tensor_mul(vsc[:, 0:1], FXB[:, ep:ep + 1], rz[:, 0:1])
                    # refresh drain row for the new epoch
                    nc.scalar.mul(d0row[:, 0:TC], ones[:, 0:TC], DT[:, ep:ep + 1])

                # select lookahead
                if sig + LA < SIG:
                    issue_selects(sig + LA)

                # init slot: halo (written at sig-2) with event fixes
                if sig == 0:
                    nc.vector.memset(cur[:, 0:1], 0.0)
                    nc.vector.memset(cur[0:32, 0:1], 1.0)
                else:
                    hsrc = haloSB[:, sig % 2:sig % 2 + 1]
                    if sig % R in (0, 1) and sig >= R:
                        nc.scalar.activation(cur[:, 0:1], hsrc, AF.Copy,
                                             scale=vsc[:, 0:1])
                    else:
                        nc.scalar.activation(cur[:, 0:1], hsrc, AF.Copy)

                # u = (b * d) * prv ; slot 0 of u stays 0
                nc.vector.scalar_tensor_tensor(
                    u[:, 0:TC], bps[sig][:, 0:TC], DT[:, ep:ep + 1],
                    prv[:, 0:TC], AL.mult, AL.mult)
                # scan
                nc.vector.tensor_tensor_scan(
                    cur[:, 1:TC + 1], d0row[:, 0:TC], u[:, 0:TC],
                    cur[:, 0:1], AL.mult, AL.add)

                # halo extraction for sig+2 (partition-shift by +32 via DMA)
                if sig + 2 < SIG:
                    nc.sync.dma_start(
                        out=haloSB[32:128, sig % 2:sig % 2 + 1],
                        in_=cur[0:96, TC:TC + 1])

                # chunk-3 output staging
                s3 = sig - SKEW * (NC - 1)
                if 0 <= s3 <= NS:
                    nc.scalar.copy(outst[96:128, s3:s3 + 1],
                                   cur[96:128, TC:TC + 1])

        nc.sync.dma_start(out=out_d.ap()[:, :], in_=outst[96:128, 0:512])
        nc.sync.dma_start(out=rz_d.ap()[:, :], in_=rzst[:, :])

    if split_waits:
        _split_multi_waits(nc)

    _CACHE[key] = nc
    _CACHE["names"] = dict(ins=["xtab", "wtab", "mats", "sched", "iot"],
                           out=["outst", "rzst"])
    return nc, _CACHE["names"]


# ---------------------------------------------------------------- host ------

def host_prep(x, seqs):
    f32, f16 = np.float32, np.float16
    BETA = beta_schedule()
    x = np.asarray(x)
    seqs = np.asarray(seqs)

    # schedule tables, per partition p = 32c + b
    dt_t = np.zeros((128, NEP), f32)
    dbn_t = np.zeros((128, NEP), f32)
    dbi_t = np.zeros((128, NEP), f32)
    fxb_t = np.ones((128, NEP), f32)
    for ep in range(NEP):
        db = BETA[:, ep] - (BETA[:, ep - 1] if ep > 0 else BETA[:, 0])
        cum = np.concatenate([[0.0], np.cumsum(db)[:-1]])    # sum_{c'<c}
        dsum = np.cumsum(db)                                  # sum_{c'<=c}
        for c in range(NC):
            p = slice(32 * c, 32 * c + 32)
            dt_t[p, ep] = np.exp(-BETA[c, ep])
            dbn_t[p, ep] = -db[c]
            dbi_t[p, ep] = -TC * cum[c]
            fxb_t[p, ep] = np.exp(-TC * dsum[c - 1]) if c >= 1 else 1.0
    sched = np.concatenate([dt_t, dbn_t, dbi_t, fxb_t], axis=1)

    iot = np.broadcast_to(np.arange(512, dtype=f32), (128, 512)).copy()

    mats = np.zeros((128, 256), f32)
    for q in range(96):
        mats[q, 32 + q] = 1.0            # perm: out[p] = in[p-32]
    for pi in range(128):
        for c in range(NC):
            mats[pi, 128 + ((pi % 32) + 32 * c) % 128] = 0.0  # placeholder
    # gsum: out[po] = sum_c in[32c + (po%32)]
    gs = np.zeros((128, 128), f32)
    for po in range(128):
        for c in range(NC):
            gs[32 * c + (po % 32), po] = 1.0
    mats[:, 128:256] = gs

    in_maps = []
    for m in range(M):
        xb = x[:, B * m:B * (m + 1), :].astype(f32)           # (NT, 32, 5)
        E = np.exp(xb[:, :, :4] - xb[:, :, 4:5])              # (NT, 32, 4)
        xt = np.empty((128, NC * TC), f16)
        for c in range(NC):
            blk = E[500 * c:500 * (c + 1)]                    # (500, 32, 4)
            for k in range(4):
                xt[32 * k:32 * k + 32, TC * c:TC * (c + 1)] = \
                    blk[:, :, k].T.astype(f16)
        sq = seqs[B * m:B * (m + 1)]                          # (32, NS)
        wt = np.zeros((128, (NS + 1) * 32), f16)
        for s in range(1, NS + 1):
            k = sq[:, s - 1]                                  # (32,)
            wt[32 * k + np.arange(B), 32 * s + np.arange(B)] = 1.0
        in_maps.append({"xtab": xt, "wtab": wt, "mats": mats,
                        "sched": sched, "iot": iot})
    return in_maps


def host_post(x, seqlens, outs, rzs):
    BETA = beta_schedule()
    x = np.asarray(x)
    seqlens = np.asarray(seqlens)
    loss = np.zeros((NB, 1), np.float32)
    for m in range(M):
        raw = outs[m][:, 0:NS + 1].astype(np.float64)         # (32, 501)
        rzv = rzs[m][0:32, :].astype(np.float64)              # rz per batch,epoch
        lnrz = np.zeros((32, SIG))
        for ep in range(1, NEP):
            sg = ep * R
            if sg < SIG:
                lnrz[:, sg] = np.log(np.maximum(rzv[:, ep], 1e-300))
        lnrz_cum = np.cumsum(lnrz, axis=1)
        C2000 = x[:, B * m:B * (m + 1), 4].sum(axis=0, dtype=np.float64)
        fwd = np.zeros((32, NS + 1))
        for s in range(NS + 1):
            sig = s + SKEW * (NC - 1)
            ep = sig // R
            corr = TC * BETA[:, ep].sum() - lnrz_cum[:, sig]
            fwd[:, s] = np.log(np.maximum(raw[:, s], 1e-300)) + corr + C2000
        sl = seqlens[B * m:B * (m + 1)].astype(np.int64)
        loss[B * m:B * (m + 1), 0] = \
            (-np.take_along_axis(fwd, sl[:, None], axis=1) / NT)[:, 0]
    return loss.astype(np.float32)


def kernel(x, seqs, seqlens):
    nc, names = build_module()
    in_maps = host_prep(x, seqs)
    res = run_bass_kernel_spmd(nc, in_maps, list(range(M)))
    outs = [res.results[m]["outst"] for m in range(M)]
    rzs = [res.results[m]["rzst"] for m in range(M)]
    return host_post(x, seqlens, outs, rzs)
